# revision 35
# baseline (speedup 1.0000x reference)
"""LCNN forward (nn_LCNN_79688823210661) on Trainium2 via Bass/Tile.

out = get_W(E @ U), E = exp(-B) (3-term Taylor; ||B|| ~ 1e-6 by construction),
B from a gauge-equivariant conv (omega) + bilinear (alpha) + ReTr act + beta.

v2 layout: 2 cores, one per batch b; each core holds the full l1-ring of its
b (32 tiles of 128 sites: tile = 4*l1 + l2//2, partition = (l2%2)*64+l3*8+l4)
and computes everything locally -- no collectives.  U ships as fp16, outputs
return as fp16 (the 2e-2 rel-err gate leaves ample margin; final accuracy is
dominated by U rounding since E ~ I + 1e-6).  The big constant operands
(permutation matrices, omega/alpha scatter matrices) are built ON DEVICE from
a few KB of indices + raw weights (is_equal masks + ones-row broadcast
matmuls), which cuts per-call host->device traffic ~5x.  kernel.py also
enables the persistent jax compilation cache and memoizes the BIR JSON so
warm calls skip the per-call re-lowering the fresh-closure jit would
otherwise pay.
"""

import os
import tempfile
from contextlib import ExitStack

import numpy as np

B, L, D, NC = 2, 8, 4, 3
NK, NOUT, NCH, NVAR = 3, 8, 8, 21
NCORES = 2
NST = 32

_CACHE = {}

# enable the persistent compilation cache: the fresh-closure jit inside
# run_bass_via_pjrt re-lowers every call; with this cache the XLA/neuronx
# backend compile is skipped on every call after the first.
try:
    import jax

    jax.config.update(
        "jax_compilation_cache_dir",
        os.path.join(tempfile.gettempdir(), "jax_cache_lcnn"),
    )
    jax.config.update("jax_persistent_cache_min_entry_size_bytes", -1)
    jax.config.update("jax_persistent_cache_min_compile_time_secs", 0.0)
except Exception:
    pass


# ----------------------------------------------------------------- host maps
def _site_maps():
    l1, l2, l3, l4 = np.meshgrid(
        np.arange(L), np.arange(L), np.arange(L), np.arange(L), indexing="ij"
    )
    t = 4 * l1 + l2 // 2
    p = (l2 % 2) * 64 + l3 * 8 + l4
    return p.reshape(-1), t.reshape(-1)


IDX_P, IDX_T = _site_maps()

ET = ((np.arange(9) % 3) * 3 + np.arange(9) // 3)  # 3x3 transpose of flat idx

MK_LIST = [(m, kk) for m in range(D) for kk in (0, 2)]

PERM_KEYS = [("l2s", 1), ("l2n", 1), ("l2s", -1), ("l2n", -1),
             (2, 1), (2, 2), (2, 4), (2, -1),
             (3, 1), (3, 2), (3, 4), (3, -1), ("id", 0)]
PERM_SLOT = {k: i for i, k in enumerate(PERM_KEYS)}

# fvec column layout (free-axis index rows, broadcast on device)
_F = {}


def _build_fvec_cols():
    cols = []

    def add(name, v):
        _F[name] = (len(np.concatenate(cols)) if cols else 0, len(v))
        cols.append(np.asarray(v, np.float32))

    f144 = np.arange(144)
    add("f144_m18", f144 % 18)
    add("dmask144", np.isin(f144 % 18, (0, 4, 8)).astype(np.float32))
    f = np.arange(128)
    add("f128", f)
    add("f_d8", f // 8)
    add("f_d64", f // 64)
    add("f_m8", f % 8)
    add("f_l3", (f % 64) // 8)
    for s in (1, 2, 4, -1):
        add(f"l4s_{s}", (f % 8 + s) % 8)
    for s in (1, 2, 4, -1):
        add(f"l3s_{s}", ((f % 64) // 8 + s) % 8)
    add("fch1", f // 18)
    add("fch2", (np.arange(52) + 128) // 18)
    add("alpha_jk_i", np.zeros(512))   # filled per-call
    add("betav", np.zeros(32))         # filled per-call
    return np.concatenate(cols)


FVEC_BASE = _build_fvec_cols()
NF = len(FVEC_BASE)

# vrow column layout (per-partition index columns)
_V = {}


def _build_vrow():
    p = np.arange(128)
    cols = []

    def add(name, v):
        _V[name] = len(cols)
        cols.append(np.asarray(v, np.float32))

    add("p", p)
    add("p_m18", p % 18)
    add("p_m64", p - 64)
    add("p_p64", p + 64)
    add("p_d8", p // 8)
    add("p_d64", p // 64)
    add("p_m8", p % 8)
    add("p_l3", (p % 64) // 8)
    for c in range(9):
        add(f"p18c_{c}", (128 * c + p) % 18)
    q = p % 18
    add("vte1", (q // 9) * 9 + ET[q % 9])
    add("sgn1", 1.0 - 2.0 * (q // 9))
    q2 = (p + 128) % 18
    add("vte2", (q2 // 9) * 9 + ET[q2 % 9])
    add("sgn2", 1.0 - 2.0 * (q2 // 9))
    return np.stack(cols, axis=1)


VROW = _build_vrow()
NV = VROW.shape[1]


# ----------------------------------------------------------------- device
def build_program():
    import concourse.bass as bass
    import concourse.bacc as bacc
    import concourse.mybir as mybir
    import concourse.tile as tile

    fp32 = mybir.dt.float32
    fp16 = mybir.dt.float16
    AX = mybir.AxisListType
    ALU = mybir.AluOpType
    ACT = mybir.ActivationFunctionType

    nc = bacc.Bacc("TRN2", target_bir_lowering=False, num_devices=NCORES)

    d_u16r = nc.dram_tensor("u16r", [128, NST, 36], fp16, kind="ExternalInput")
    d_u16i = nc.dram_tensor("u16i", [128, NST, 36], fp16, kind="ExternalInput")
    d_vrow = nc.dram_tensor("vrow", [128, NV], fp32, kind="ExternalInput")
    d_fvec = nc.dram_tensor("fvec", [1, NF], fp32, kind="ExternalInput")
    d_wdir = nc.dram_tensor("wdir", [10, 72], fp32, kind="ExternalInput")
    d_wdag = nc.dram_tensor("wdag", [10, 72], fp32, kind="ExternalInput")
    d_wcon = nc.dram_tensor("wcon", [1, 72], fp32, kind="ExternalInput")
    d_ab = nc.dram_tensor("ab", [9, 128, 8], fp32, kind="ExternalInput")
    # 120 floats/site: 10 ch x re/im x first-2-rows; host reconstructs row 3
    d_out = nc.dram_tensor("wout", [128, NST, 120], fp16, kind="ExternalOutput")

    DBG = bool(os.environ.get("KDBG"))
    dbg_t = {}

    def dbg(name, shape):
        if name not in dbg_t:
            dbg_t[name] = nc.dram_tensor(name, shape, fp32, kind="ExternalOutput")
        return dbg_t[name]

    es = ExitStack()
    with tile.TileContext(nc) as tc:
        sb = es.enter_context(tc.tile_pool(name="sb", bufs=1))
        scr = es.enter_context(tc.tile_pool(name="scr", bufs=1))
        psp = es.enter_context(tc.tile_pool(name="psp", bufs=3, space="PSUM"))
        pst = es.enter_context(tc.tile_pool(name="pst", bufs=2, space="PSUM"))

        # ---------------- load + build constants ----------------
        vrow = sb.tile([128, NV], fp32)
        nc.sync.dma_start(vrow[:], d_vrow.ap())

        def drain(dst_ap, src_ap):
            nc.scalar.activation(dst_ap, src_ap, ACT.Copy)

        ones1 = sb.tile([1, 128], fp32)
        nc.gpsimd.memset(ones1[:], 1.0)

        FB = sb.tile([128, NF], fp32)
        with ExitStack() as ph:
            pfv = ph.enter_context(tc.tile_pool(name="pfv", bufs=1))
            fvec = pfv.tile([1, NF], fp32)
            nc.sync.dma_start(fvec[:], d_fvec.ap())
            for off in range(0, NF, 512):
                w = min(512, NF - off)
                pt = psp.tile([128, 512], fp32, tag="mm")
                nc.tensor.matmul(pt[:, 0:w], ones1[:], fvec[:, off:off + w],
                                 start=True, stop=True)
                drain(FB[:, off:off + w], pt[:, 0:w])

        def fb(name, n=128):
            off, ln = _F[name]
            return FB[0:n, off:off + ln]

        def vc(name, n=128):
            return vrow[0:n, _V[name]:_V[name] + 1]

        # permutation matrices [128 part=k(in row), 13, 128 f=m(out row)]
        perm = sb.tile([128, len(PERM_KEYS), 128], fp32)

        def P(key):
            return perm[:, PERM_SLOT[key], :]

        tA = scr.tile([128, 128], fp32, tag="pb_a")
        tB = scr.tile([128, 128], fp32, tag="pb_b")
        for key in PERM_KEYS:
            dst = P(key)
            if key == ("id", 0):
                nc.vector.tensor_tensor(
                    dst, vc("p").broadcast_to([128, 128]), fb("f128"),
                    ALU.is_equal)
            elif key[0] in ("l2s", "l2n"):
                up = (key == ("l2s", 1)) or (key == ("l2n", -1))
                col = "p_m64" if up else "p_p64"
                nc.vector.tensor_tensor(
                    dst, vc(col).broadcast_to([128, 128]), fb("f128"),
                    ALU.is_equal)
            elif key[0] == 3:   # l4 shift
                nc.vector.tensor_tensor(
                    tA[:], vc("p_d8").broadcast_to([128, 128]), fb("f_d8"),
                    ALU.is_equal)
                nc.vector.tensor_tensor(
                    tB[:], vc("p_m8").broadcast_to([128, 128]),
                    fb(f"l4s_{key[1]}"), ALU.is_equal)
                nc.vector.tensor_tensor(dst, tA[:], tB[:], ALU.mult)
            else:               # l3 shift
                nc.vector.tensor_tensor(
                    tA[:], vc("p_d64").broadcast_to([128, 128]), fb("f_d64"),
                    ALU.is_equal)
                nc.vector.tensor_tensor(
                    tB[:], vc("p_m8").broadcast_to([128, 128]), fb("f_m8"),
                    ALU.is_equal)
                nc.vector.tensor_tensor(tA[:], tA[:], tB[:], ALU.mult)
                nc.vector.tensor_tensor(
                    tB[:], vc("p_l3").broadcast_to([128, 128]),
                    fb(f"l3s_{key[1]}"), ALU.is_equal)
                nc.vector.tensor_tensor(dst, tA[:], tB[:], ALU.mult)
        if DBG:
            nc.sync.dma_start(dbg("dbg_perm", [128, 13, 128]).ap(), perm[:])

        # U fp16 -> fp32
        ure = sb.tile([128, NST, 36], fp32)
        uim = sb.tile([128, NST, 36], fp32)
        with ExitStack() as ph:
            pl16 = ph.enter_context(tc.tile_pool(name="p16", bufs=1))
            u16r = pl16.tile([128, NST, 36], fp16)
            u16i = pl16.tile([128, NST, 36], fp16)
            nc.sync.dma_start(u16r[:], d_u16r.ap())
            nc.sync.dma_start(u16i[:], d_u16i.ap())
            nc.scalar.activation(ure[:], u16r[:], ACT.Copy)
            nc.scalar.activation(uim[:], u16i[:], ACT.Copy)

        # long-lived intermediates (kept across the WE/scatter pool stack)
        pcd = ExitStack()
        pcw = pcd.enter_context(tc.tile_pool(name="pcw", bufs=1))
        Wc = pcw.tile([128, NST, 8, 2, 9], fp32)
        Wb = pcw.tile([128, NST, 8, 2, 9], fp32)
        eur = pcw.tile([128, NST, 36], fp32)
        eui = pcw.tile([128, NST, 36], fp32)

        # scatter matrices for omega conv / alpha bilinear (fp16: conv
        # precision only perturbs the output at the 1e-6 scale of E-I)
        esbd = ExitStack()
        pbd = esbd.enter_context(tc.tile_pool(name="pbd", bufs=1))
        pwe = esbd.enter_context(tc.tile_pool(name="pwe", bufs=1))
        pwin = esbd.enter_context(tc.tile_pool(name="pwin", bufs=4))
        wsc1 = pbd.tile([128, 9, 144], fp16)
        wsc2 = pbd.tile([52, 9, 144], fp16)
        wsc3 = pbd.tile([1, 9, 144], fp16)
        asc = pbd.tile([128, 9, 144], fp16)
        with ExitStack() as ph:
            pw = ph.enter_context(tc.tile_pool(name="pw", bufs=1))
            wdir = pw.tile([10, 72], fp32)
            wdag = pw.tile([10, 72], fp32)
            wcon = pw.tile([1, 72], fp32)
            ab = pw.tile([128, 9, 8], fp32)
            nc.sync.dma_start(wdir[:], d_wdir.ap())
            nc.sync.dma_start(wdag[:], d_wdag.ap())
            nc.sync.dma_start(wcon[:], d_wcon.ap())
            nc.sync.dma_start(ab[:], d_ab.ap().rearrange("n p f -> p n f"))

            oh1 = pw.tile([10, 128], fp32)
            oh2 = pw.tile([10, 52], fp32)
            nc.vector.tensor_tensor(oh1[:], vc("p", 10).broadcast_to([10, 128]),
                                    fb("fch1", 10), ALU.is_equal)
            nc.vector.tensor_tensor(oh2[:], vc("p", 10).broadcast_to([10, 52]),
                                    fb("fch2", 10), ALU.is_equal)
            bd1 = pw.tile([128, 9, 8], fp32)
            bg1 = pw.tile([128, 9, 8], fp32)
            bd2 = pw.tile([52, 9, 8], fp32)
            bg2 = pw.tile([52, 9, 8], fp32)
            for (oh, n, src, dst) in ((oh1, 128, wdir, bd1), (oh1, 128, wdag, bg1),
                                      (oh2, 52, wdir, bd2), (oh2, 52, wdag, bg2)):
                pt = psp.tile([128, 512], fp32, tag="mm")
                nc.tensor.matmul(pt[0:n, 0:72], oh[:], src[:],
                                 start=True, stop=True)
                drain(dst[:].rearrange("p a b -> p (a b)"), pt[0:n, 0:72])

            meq1 = pw.tile([128, 144], fp32)
            mt1 = pw.tile([128, 144], fp32)
            meq2 = pw.tile([52, 144], fp32)
            mt2 = pw.tile([52, 144], fp32)
            nc.vector.tensor_tensor(meq1[:], vc("p_m18").broadcast_to([128, 144]),
                                    fb("f144_m18"), ALU.is_equal)
            nc.vector.tensor_tensor(mt1[:], fb("f144_m18"),
                                    vc("vte1").broadcast_to([128, 144]),
                                    ALU.is_equal)
            nc.vector.tensor_tensor(mt1[:], mt1[:],
                                    vc("sgn1").broadcast_to([128, 144]), ALU.mult)
            nc.vector.tensor_tensor(meq2[:],
                                    vc("p18c_1", 52).broadcast_to([52, 144]),
                                    fb("f144_m18", 52), ALU.is_equal)
            nc.vector.tensor_tensor(mt2[:], fb("f144_m18", 52),
                                    vc("vte2", 52).broadcast_to([52, 144]),
                                    ALU.is_equal)
            nc.vector.tensor_tensor(mt2[:], mt2[:],
                                    vc("sgn2", 52).broadcast_to([52, 144]),
                                    ALU.mult)
            ta1 = pw.tile([128, 144], fp32)
            tb1 = pw.tile([128, 144], fp32)
            for s in range(9):
                for (n, meq, mt, bd, bg, out) in (
                        (128, meq1, mt1, bd1, bg1, wsc1),
                        (52, meq2, mt2, bd2, bg2, wsc2)):
                    va = ta1[0:n].rearrange("p (a b) -> p a b", a=8)
                    vb = tb1[0:n].rearrange("p (a b) -> p a b", a=8)
                    nc.vector.tensor_tensor(
                        va, meq[:].rearrange("p (a b) -> p a b", a=8),
                        bd[:, s, :].unsqueeze(2).broadcast_to([n, 8, 18]),
                        ALU.mult)
                    nc.vector.tensor_tensor(
                        vb, mt[:].rearrange("p (a b) -> p a b", a=8),
                        bg[:, s, :].unsqueeze(2).broadcast_to([n, 8, 18]),
                        ALU.mult)
                    nc.vector.tensor_tensor(out[:, s, :], ta1[0:n], tb1[0:n],
                                            ALU.add)
                nc.vector.tensor_tensor(
                    wsc3[:, s, :].rearrange("p (a b) -> p a b", a=8),
                    fb("dmask144", 1).rearrange("p (a b) -> p a b", a=8),
                    wcon[:, 8 * s:8 * s + 8].unsqueeze(2)
                    .broadcast_to([1, 8, 18]),
                    ALU.mult)
            # alpha scatter: 9 chunks of 128 rows
            for c in range(9):
                nc.vector.tensor_tensor(
                    ta1[:], vc(f"p18c_{c}").broadcast_to([128, 144]),
                    fb("f144_m18"), ALU.is_equal)
                nc.vector.tensor_tensor(
                    asc[:, c, :].rearrange("p (a b) -> p a b", a=8),
                    ta1[:].rearrange("p (a b) -> p a b", a=8),
                    ab[:, c, :].unsqueeze(2).broadcast_to([128, 8, 18]),
                    ALU.mult)
        if DBG:
            nc.sync.dma_start(dbg("dbg_wsc1", [128, 9, 144]).ap(), wsc1[:])
            nc.sync.dma_start(dbg("dbg_wsc2", [52, 9, 144]).ap(), wsc2[:])
            nc.sync.dma_start(dbg("dbg_wsc3", [1, 9, 144]).ap(), wsc3[:])
            nc.sync.dma_start(dbg("dbg_asc", [128, 9, 144]).ap(), asc[:])

        WEc = pbd.tile([1, 128], fp16)
        nc.gpsimd.memset(WEc[:], 1.0)

        # ---------------- helpers ----------------
        def pe_apply(dst_flat, terms):
            """dst_flat [128, n] <- sum_i perm_key_i(src_flat_i)."""
            n = dst_flat.shape[1]
            for off in range(0, n, 512):
                w = min(512, n - off)
                pt = psp.tile([128, 512], fp32, tag="mm")
                for i, (key, src) in enumerate(terms):
                    nc.tensor.matmul(pt[:, 0:w], P(key), src[:, off:off + w],
                                     start=(i == 0), stop=(i == len(terms) - 1))
                drain(dst_flat[:, off:off + w], pt[:, 0:w])

        MAXG = 64

        def cmm(Gdims, Are, Aim, Bre, Bim, outre, outim,
                conj_a=False, conj_b=False, acc=False):
            """C = opA(A) @ opB(B); operands [128, *Gdims, 9]; 3x3 complex.

            opX = conjugate-transpose when conj_x.  acc: add into out.
            """
            Gflat = int(np.prod(Gdims))
            assert Gflat <= MAXG, Gdims
            assert len(Gdims) == 1, Gdims
            ca = scr.tile([128, MAXG, 9], fp32, tag="c_ca")
            cb2 = scr.tile([128, MAXG, 9], fp32, tag="c_cb2")
            cb3 = scr.tile([128, MAXG, 9], fp32, tag="c_cb3")
            k1 = scr.tile([128, MAXG, 3, 3, 3], fp32, tag="c_k1")
            k2 = scr.tile([128, MAXG, 3, 3, 3], fp32, tag="c_k2")
            k3 = scr.tile([128, MAXG, 3, 3, 3], fp32, tag="c_k3")
            sre, sim = k3, k2   # combines overwrite in place

            G = Gflat

            def gv(t):
                return t[:, 0:G, :]

            nc.vector.tensor_tensor(gv(ca), Are, Aim,
                                    ALU.subtract if conj_a else ALU.add)
            if not conj_b:
                nc.vector.tensor_tensor(gv(cb2), Bim, Bre, ALU.subtract)
                nc.vector.tensor_tensor(gv(cb3), Bre, Bim, ALU.add)
            else:
                nc.vector.tensor_tensor(gv(cb2), Bre, Bim, ALU.add)
                nc.vector.tensor_tensor(gv(cb3), Bre, Bim, ALU.subtract)

            def arow(t, aa):
                if conj_a:
                    v = t[:, :, aa:9:3] if t.ndim == 3 else None
                else:
                    v = t[:, :, 3 * aa:3 * aa + 3]
                return v.unsqueeze(2).broadcast_to([128, G, 3, 3])

            def bfull(t):
                if conj_b:
                    return t.rearrange("p G (b c) -> p G b c", b=3)
                return t.rearrange("p G (c b) -> p G b c", c=3)

            yr = bfull(Bre)
            y2 = bfull(gv(cb2))
            y3 = bfull(gv(cb3))
            for aa in range(3):
                nc.vector.tensor_tensor(k1[:, 0:G, aa, :, :], yr,
                                        arow(gv(ca), aa), ALU.mult)
                nc.vector.tensor_tensor(k2[:, 0:G, aa, :, :], arow(Are, aa),
                                        y2, ALU.mult)
                nc.vector.tensor_tensor(k3[:, 0:G, aa, :, :], arow(Aim, aa),
                                        y3, ALU.mult)

            def kf(t):
                return t[:, 0:G, :, :, :].rearrange("p G a b c -> p G (a b c)")

            nc.vector.tensor_tensor(kf(sre), kf(k1), kf(k3),
                                    ALU.add if conj_a else ALU.subtract)
            nc.vector.tensor_tensor(kf(sim), kf(k1), kf(k2),
                                    ALU.subtract if conj_b else ALU.add)

            def c3(t, cc):
                return t[:, 0:G, :, :, cc]

            if acc:
                # ca/cb3 are dead by now; reuse them as the reduce scratch
                tre = ca[:].rearrange("p g (a b) -> p g a b", a=3)
                tim = cb3[:].rearrange("p g (a b) -> p g a b", a=3)
                for (tt, ss, oo) in ((tre, sre, outre), (tim, sim, outim)):
                    nc.vector.tensor_tensor(tt[:, 0:G], c3(ss, 0), c3(ss, 1),
                                            ALU.add)
                    nc.vector.tensor_tensor(tt[:, 0:G], tt[:, 0:G], c3(ss, 2),
                                            ALU.add)
                    nc.vector.tensor_tensor(oo, oo, tt[:, 0:G], ALU.add)
            else:
                for (ss, oo) in ((sre, outre), (sim, outim)):
                    nc.vector.tensor_tensor(oo, c3(ss, 0), c3(ss, 1), ALU.add)
                    nc.vector.tensor_tensor(oo, oo, c3(ss, 2), ALU.add)

        def uslc(t, t0, n, mu):
            return t[:, t0:t0 + n, 9 * mu:9 * mu + 9]

        def ring_chunks(wt0, nwt, shift_tiles):
            out = []
            t0 = wt0
            end = wt0 + nwt
            while t0 < end:
                b0 = (t0 + shift_tiles) % NST
                n = min(end - t0, NST - b0)
                out.append((t0, n, b0))
                t0 += n
            return out

        def roll_l2(dst, src, sign):
            """dst [128, 32, F] <- src rolled by sign*e_l2 (+1/-1)."""
            F = src.shape[2]
            ks, kn = ("l2s", sign), ("l2n", sign)
            for s in range(NST // 4):
                b0 = 4 * s
                pt = psp.tile([128, 512], fp32, tag="mm")
                n4 = 4 * F
                nc.tensor.matmul(pt[:, 0:n4], P(ks),
                                 src[:, b0:b0 + 4, :].rearrange("p t f -> p (t f)"),
                                 start=True, stop=False)
                if sign == 1:
                    nc.tensor.matmul(pt[:, 0:3 * F], P(kn),
                                     src[:, b0 + 1:b0 + 4, :].rearrange("p t f -> p (t f)"),
                                     start=False, stop=False)
                    nc.tensor.matmul(pt[:, 3 * F:n4], P(kn),
                                     src[:, b0:b0 + 1, :].rearrange("p t f -> p (t f)"),
                                     start=False, stop=True)
                else:
                    nc.tensor.matmul(pt[:, F:n4], P(kn),
                                     src[:, b0:b0 + 3, :].rearrange("p t f -> p (t f)"),
                                     start=False, stop=False)
                    nc.tensor.matmul(pt[:, 0:F], P(kn),
                                     src[:, b0 + 3:b0 + 4, :].rearrange("p t f -> p (t f)"),
                                     start=False, stop=True)
                drain(dst[:, 4 * s:4 * s + 4, :].rearrange("p t f -> p (t f)"),
                      pt[:, 0:n4])

        # ---------------- get_W (full ring, 32 tiles) ----------------
        def emit_getw(u_re, u_im, ws):
            """ws [128, 32, 10, 2, 9] <- W channels on all ring tiles."""
            with ExitStack() as ph:
                pl = ph.enter_context(tc.tile_pool(name="getw", bufs=1))
                urr = pl.tile([128, 3, NST, 36], fp32, tag="urr")
                uri = pl.tile([128, 3, NST, 36], fp32, tag="uri")
                for (src, dst) in ((u_re, urr), (u_im, uri)):
                    roll_l2(dst[:, 0, :, :], src, 1)
                    for axi, key in ((1, (2, 1)), (2, (3, 1))):
                        pe_apply(dst[:, axi, :, :].rearrange("p t f -> p (t f)"),
                                 [(key, src[:].rearrange("p t f -> p (t f)"))])

                # three shared [128, NST, 2, 9] scratch buffers
                bufA = pl.tile([128, NST, 2, 9], fp32, tag="bufA")
                bufB = pl.tile([128, NST, 2, 9], fp32, tag="bufB")
                bufC = pl.tile([128, NST, 2, 9], fp32, tag="bufC")

                # plaquettes pairwise: P_mn = D_mn D_nm^+,
                # D_ab = U_a(x) U_b(x+e_a)
                Dmn, Dnm = bufA, bufB

                def dfill(dst, a, b_):
                    if a == 0:
                        for (t0, n, b0) in ring_chunks(0, NST, 4):
                            cmm((n,), uslc(u_re, t0, n, 0), uslc(u_im, t0, n, 0),
                                uslc(u_re, b0, n, b_), uslc(u_im, b0, n, b_),
                                dst[:, t0:t0 + n, 0, :], dst[:, t0:t0 + n, 1, :])
                    else:
                        cmm((32,), uslc(u_re, 0, NST, a), uslc(u_im, 0, NST, a),
                            urr[:, a - 1, :, 9 * b_:9 * b_ + 9],
                            uri[:, a - 1, :, 9 * b_:9 * b_ + 9],
                            dst[:, :, 0, :], dst[:, :, 1, :])

                for ch, (m, nn) in enumerate(
                        ((0, 1), (0, 2), (0, 3), (1, 2), (1, 3), (2, 3))):
                    dfill(Dmn, m, nn)
                    dfill(Dnm, nn, m)
                    o = ws[:, :, ch, :, :]
                    for g0 in (0, 16):
                        cmm((16,), Dmn[:, g0:g0 + 16, 0, :], Dmn[:, g0:g0 + 16, 1, :],
                            Dnm[:, g0:g0 + 16, 0, :], Dnm[:, g0:g0 + 16, 1, :],
                            o[:, g0:g0 + 16, 0, :], o[:, g0:g0 + 16, 1, :],
                            conj_b=True)

                # Polyakov mu=0 on the ring (l1): +1 slab = +4 tiles
                P20, P40 = bufA, bufB
                for (t0, n, b0) in ring_chunks(0, NST, 4):
                    for c0 in range(0, n, 32):
                        cn = min(32, n - c0)
                        cmm((cn,), uslc(u_re, t0 + c0, cn, 0),
                            uslc(u_im, t0 + c0, cn, 0),
                            uslc(u_re, b0 + c0, cn, 0), uslc(u_im, b0 + c0, cn, 0),
                            P20[:, t0 + c0:t0 + c0 + cn, 0, :],
                            P20[:, t0 + c0:t0 + c0 + cn, 1, :])
                for (t0, n, b0) in ring_chunks(0, NST, 8):
                    cmm((n,), P20[:, t0:t0 + n, 0, :], P20[:, t0:t0 + n, 1, :],
                        P20[:, b0:b0 + n, 0, :], P20[:, b0:b0 + n, 1, :],
                        P40[:, t0:t0 + n, 0, :], P40[:, t0:t0 + n, 1, :])
                for (t0, n, b0) in ring_chunks(0, NST, 16):
                    cmm((n,), P40[:, t0:t0 + n, 0, :], P40[:, t0:t0 + n, 1, :],
                        P40[:, b0:b0 + n, 0, :], P40[:, b0:b0 + n, 1, :],
                        ws[:, t0:t0 + n, 6, 0, :], ws[:, t0:t0 + n, 6, 1, :])

                # Polyakov mu=1 (l2): +2 l2 = +1 tile in slab (wrap mod 4)
                P2m, P4m, tsh = bufA, bufB, bufC
                cmm((32,), uslc(u_re, 0, NST, 1), uslc(u_im, 0, NST, 1),
                    urr[:, 0, :, 9:18], uri[:, 0, :, 9:18],
                    P2m[:, :, 0, :], P2m[:, :, 1, :])

                def slab_shift(dst, src, sh):
                    sv = src[:].rearrange("p (s t) r e -> p s t r e", t=4)
                    dv = dst[:].rearrange("p (s t) r e -> p s t r e", t=4)
                    nc.sync.dma_start(dv[:, :, 0:4 - sh], sv[:, :, sh:4])
                    nc.sync.dma_start(dv[:, :, 4 - sh:4], sv[:, :, 0:sh])

                slab_shift(tsh, P2m, 1)
                cmm((32,), P2m[:, :, 0, :], P2m[:, :, 1, :],
                    tsh[:, :, 0, :], tsh[:, :, 1, :],
                    P4m[:, :, 0, :], P4m[:, :, 1, :])
                slab_shift(tsh, P4m, 2)
                cmm((32,), P4m[:, :, 0, :], P4m[:, :, 1, :],
                    tsh[:, :, 0, :], tsh[:, :, 1, :],
                    ws[:, :, 7, 0, :], ws[:, :, 7, 1, :])

                # Polyakov mu=2 (l3) / mu=3 (l4)
                for mu, axi, ax in ((2, 1, 2), (3, 2, 3)):
                    cmm((32,), uslc(u_re, 0, NST, mu), uslc(u_im, 0, NST, mu),
                        urr[:, axi, :, 9 * mu:9 * mu + 9],
                        uri[:, axi, :, 9 * mu:9 * mu + 9],
                        P2m[:, :, 0, :], P2m[:, :, 1, :])
                    pe_apply(tsh[:].rearrange("p t r e -> p (t r e)"),
                             [((ax, 2), P2m[:].rearrange("p t r e -> p (t r e)"))])
                    cmm((32,), P2m[:, :, 0, :], P2m[:, :, 1, :],
                        tsh[:, :, 0, :], tsh[:, :, 1, :],
                        P4m[:, :, 0, :], P4m[:, :, 1, :])
                    pe_apply(tsh[:].rearrange("p t r e -> p (t r e)"),
                             [((ax, 4), P4m[:].rearrange("p t r e -> p (t r e)"))])
                    o = ws[:, :, 6 + mu, :, :]
                    cmm((32,), P4m[:, :, 0, :], P4m[:, :, 1, :],
                        tsh[:, :, 0, :], tsh[:, :, 1, :],
                        o[:, :, 0, :], o[:, :, 1, :])

        # ================= phase A+B: W everywhere -> WE =================
        WE1 = pwe.tile([128, 10, 10, 10, 10], fp16)
        WE2 = pwe.tile([52, 10, 10, 10, 10], fp16)

        with ExitStack() as ph:
            pa = ph.enter_context(tc.tile_pool(name="pa", bufs=1))
            wsA = pa.tile([128, NST, 10, 2, 9], fp32)
            emit_getw(ure, uim, wsA[:])
            if DBG:
                nc.sync.dma_start(dbg("dbg_wsA", [128, NST, 180]).ap(),
                                  wsA[:].rearrange("p t c r e -> p t (c r e)"))
            wsAf = wsA[:].rearrange("p t c r e -> p t (c r e)")
            for st in range(NST):
                j, t = st // 4, st % 4
                for (r0, n, we) in ((0, 128, WE1), (128, 52, WE2)):
                    pt = pst.tile([128, 128], fp32, tag="tr")
                    nc.tensor.matmul(pt[0:n, 0:128], wsAf[:, st, r0:r0 + n],
                                     P(("id", 0)), is_transpose=True,
                                     start=True, stop=True)
                    dst = we[0:n, 1 + j, 2 * t + 1:2 * t + 3, 1:9, 1:9]
                    drain(dst, pt[0:n, 0:128].rearrange(
                        "q (l2 l3 l4) -> q l2 l3 l4", l2=2, l3=8))
        for we, n in ((WE1, 128), (WE2, 52)):
            v = we[0:n]
            drain(v[:, 1:9, 0, 1:9, 1:9], v[:, 1:9, 8, 1:9, 1:9])
            drain(v[:, 1:9, 9, 1:9, 1:9], v[:, 1:9, 1, 1:9, 1:9])
            for sl in range(1, 9):
                drain(v[:, sl, 1:9, 0, 1:9], v[:, sl, 1:9, 8, 1:9])
                drain(v[:, sl, 1:9, 9, 1:9], v[:, sl, 1:9, 1, 1:9])
                drain(v[:, sl, 1:9, 1:9, 0], v[:, sl, 1:9, 1:9, 8])
                drain(v[:, sl, 1:9, 1:9, 9], v[:, sl, 1:9, 1:9, 1])
            # l1 ring wrap: slab index 1..8 <-> l1 0..7
            drain(v[:, 0], v[:, 8])
            drain(v[:, 9], v[:, 1])

        # ================= phase C: conv (omega) + conjugation ============
        def conv_mm(scat_idx, st, dvec):
            j, t = st // 4 + 1, st % 4
            d1, d2, d3, d4 = dvec
            pt = psp.tile([128, 512], fp32, tag="mm")
            wstage1 = pwin.tile([128, 128], fp16, tag="ws1")
            wstage2 = pwin.tile([52, 128], fp16, tag="ws2")
            nc.gpsimd.tensor_copy(wstage1[:],
                                  WE1[:, j + d1, 2 * t + 1 + d2:2 * t + 3 + d2,
                                      1 + d3:9 + d3, 1 + d4:9 + d4])
            nc.gpsimd.tensor_copy(wstage2[:],
                                  WE2[:, j + d1, 2 * t + 1 + d2:2 * t + 3 + d2,
                                      1 + d3:9 + d3, 1 + d4:9 + d4])
            nc.tensor.matmul(pt[:, 0:144], wstage1[:], wsc1[:, scat_idx, :],
                             start=True, stop=False)
            nc.tensor.matmul(pt[:, 0:144], wstage2[:], wsc2[:, scat_idx, :],
                             start=False, stop=False)
            nc.tensor.matmul(pt[:, 0:144], WEc[:], wsc3[:, scat_idx, :],
                             start=False, stop=True)
            return pt

        for st in range(NST):
            pt = conv_mm(8, st, (0, 0, 0, 0))
            drain(Wc[:, st, :, :, :].rearrange("p i r e -> p (i r e)"),
                  pt[:, 0:144])

        with ExitStack() as ph:
            pc = ph.enter_context(tc.tile_pool(name="pc", bufs=1))
            urn = pc.tile([128, 3, NST, 36], fp32, tag="urn")
            uin = pc.tile([128, 3, NST, 36], fp32, tag="uin")
            for (src, dst) in ((ure, urn), (uim, uin)):
                roll_l2(dst[:, 0, :, :], src, -1)
                for axi, key in ((1, (2, -1)), (2, (3, -1))):
                    pe_apply(dst[:, axi, :, :].rearrange("p t f -> p (t f)"),
                             [(key, src[:].rearrange("p t f -> p (t f)"))])

            HN = 8
            Asb = pc.tile([128, HN, 8, 2, 9], fp32, tag="Asb")
            Zsb = pc.tile([128, HN, 8, 2, 9], fp32, tag="Zsb")
            Prep = pc.tile([128, HN, 8, 9], fp32, tag="Prep")
            Pimp = pc.tile([128, HN, 8, 9], fp32, tag="Pimp")
            fl = lambda ap: ap.rearrange("p t i e -> p (t i) e")
            for mki, (m, kk) in enumerate(MK_LIST):
                dvec = [0, 0, 0, 0]
                dvec[m] = kk - 1
                for h0 in range(0, NST, HN):
                    for si in range(HN):
                        pt = conv_mm(mki, h0 + si, tuple(dvec))
                        drain(Asb[:, si, :, :, :].rearrange("p i r e -> p (i r e)"),
                              pt[:, 0:144])
                    if kk == 2:
                        chunks = [(h0, HN, h0)]
                        ca_flag, cb_flag = False, True   # Y = U A ; Wc += Y U^+
                    else:
                        if m == 0:
                            chunks = ring_chunks(h0, HN, NST - 4)
                        else:
                            chunks = [(h0, HN, h0)]
                        ca_flag, cb_flag = True, False   # Y = V^+ A ; Wc += Y V
                    for (t0, n, b0) in chunks:
                        if kk == 2:
                            Pre = uslc(ure, t0, n, m)
                            Pim = uslc(uim, t0, n, m)
                        elif m == 0:
                            Pre = uslc(ure, b0, n, m)
                            Pim = uslc(uim, b0, n, m)
                        else:
                            Pre = urn[:, m - 1, t0:t0 + n, 9 * m:9 * m + 9]
                            Pim = uin[:, m - 1, t0:t0 + n, 9 * m:9 * m + 9]
                        o0 = t0 - h0
                        nc.vector.tensor_copy(
                            Prep[:, o0:o0 + n], Pre.unsqueeze(2)
                            .broadcast_to([128, n, 8, 9]))
                        nc.vector.tensor_copy(
                            Pimp[:, o0:o0 + n], Pim.unsqueeze(2)
                            .broadcast_to([128, n, 8, 9]))
                    cmm((64,), fl(Prep[:]), fl(Pimp[:]),
                        fl(Asb[:, :, :, 0, :]), fl(Asb[:, :, :, 1, :]),
                        fl(Zsb[:, :, :, 0, :]), fl(Zsb[:, :, :, 1, :]),
                        conj_a=ca_flag)
                    cmm((64,), fl(Zsb[:, :, :, 0, :]), fl(Zsb[:, :, :, 1, :]),
                        fl(Prep[:]), fl(Pimp[:]),
                        fl(Wc[:, h0:h0 + HN, :, 0, :]),
                        fl(Wc[:, h0:h0 + HN, :, 1, :]),
                        conj_b=cb_flag, acc=True)
        if DBG:
            nc.sync.dma_start(dbg("dbg_wc", [128, NST, 144]).ap(),
                              Wc[:].rearrange("p t i r e -> p t (i r e)"))

        # ================= phase D: bilinear (alpha) ======================
        with ExitStack() as ph:
            pd = ph.enter_context(tc.tile_pool(name="pd", bufs=1))
            Qt = pd.tile([128, 8, 8, 2, 9], fp32, tag="Qt")
            Wjr = pd.tile([128, 8, 8, 9], fp32, tag="Wjr")
            Wji = pd.tile([128, 8, 8, 9], fp32, tag="Wji")
            Wkr = pd.tile([128, 8, 8, 9], fp32, tag="Wkr")
            Wki = pd.tile([128, 8, 8, 9], fp32, tag="Wki")
            fl2 = lambda ap: ap.rearrange("p j k e -> p (j k) e")
            for oi in range(NST):
                nc.vector.tensor_copy(Wjr[:], Wc[:, oi, :, 0, :].unsqueeze(2)
                                      .broadcast_to([128, 8, 8, 9]))
                nc.vector.tensor_copy(Wji[:], Wc[:, oi, :, 1, :].unsqueeze(2)
                                      .broadcast_to([128, 8, 8, 9]))
                nc.vector.tensor_copy(Wkr[:], Wc[:, oi, :, 0, :].unsqueeze(1)
                                      .broadcast_to([128, 8, 8, 9]))
                nc.vector.tensor_copy(Wki[:], Wc[:, oi, :, 1, :].unsqueeze(1)
                                      .broadcast_to([128, 8, 8, 9]))
                cmm((64,), fl2(Wjr[:]), fl2(Wji[:]), fl2(Wkr[:]), fl2(Wki[:]),
                    fl2(Qt[:, :, :, 0, :]), fl2(Qt[:, :, :, 1, :]))
                Qf = Qt[:].rearrange("p j k r e -> p (j k r e)")
                pt2 = psp.tile([128, 512], fp32, tag="mm2")
                for cch in range(9):
                    ptr = pst.tile([128, 128], fp32, tag="tr")
                    nc.tensor.matmul(ptr[:], Qf[:, 128 * cch:128 * cch + 128],
                                     P(("id", 0)), is_transpose=True,
                                     start=True, stop=True)
                    qe = pd.tile([128, 128], fp16, tag="qe")
                    drain(qe[:], ptr[:])
                    nc.tensor.matmul(pt2[:, 0:144], qe[:], asc[:, cch, :],
                                     start=(cch == 0), stop=(cch == 8))
                drain(Wb[:, oi, :, :, :].rearrange("p i r e -> p (i r e)"),
                      pt2[:, 0:144])
        if DBG:
            nc.sync.dma_start(dbg("dbg_wb", [128, NST, 144]).ap(),
                              Wb[:].rearrange("p t i r e -> p t (i r e)"))

        esbd.close()

        # ================= phase E: act + beta + Taylor + EU ==============
        with ExitStack() as ph:
            pe = ph.enter_context(tc.tile_pool(name="pe", bufs=1))
            fsc = pe.tile([128, NST, 8], fp32, tag="fsc")
            nc.vector.tensor_reduce(fsc[:], Wb[:, :, :, 0, 0:9:4], AX.X, ALU.add)
            Wa = pe.tile([128, NST, 8, 2, 9], fp32, tag="Wa")
            fbb = fsc[:].unsqueeze(3).broadcast_to([128, NST, 8, 18])
            nc.vector.tensor_tensor(Wa[:].rearrange("p t i r e -> p t i (r e)"),
                                    Wb[:].rearrange("p t i r e -> p t i (r e)"),
                                    fbb, ALU.mult)
            Wah = pe.tile([128, NST, 8, 2, 9], fp32, tag="Wah")
            WaT = Wa[:].rearrange("p t i r (a b) -> p (t i) r b a", a=3)
            WaF = Wa[:].rearrange("p t i r e -> p (t i) r e")
            WahF = Wah[:].rearrange("p t i r e -> p (t i) r e")
            nc.vector.tensor_tensor(WahF[:, :, 0, :], WaF[:, :, 0, :],
                                    WaT[:, :, 0], ALU.subtract)
            nc.vector.tensor_tensor(WahF[:, :, 1, :], WaF[:, :, 1, :],
                                    WaT[:, :, 1], ALU.add)
            trh = pe.tile([128, NST, 8], fp32, tag="trh")
            nc.vector.tensor_reduce(trh[:], Wah[:, :, :, 1, 0:9:4], AX.X, ALU.add)
            trb = trh[:].unsqueeze(3).broadcast_to([128, NST, 8, 3])
            nc.vector.scalar_tensor_tensor(Wah[:, :, :, 1, 0:9:4], trb,
                                           -1.0 / 3.0,
                                           Wah[:, :, :, 1, 0:9:4],
                                           ALU.mult, ALU.add)
            beta_t = fb("betav")    # [128, 32] broadcast rows of beta/2
            Bm = pe.tile([128, NST, 4, 2, 9], fp32, tag="Bm")
            for m in range(4):
                for i in range(8):
                    sc = beta_t[:, 8 * m + i:8 * m + i + 1]
                    src = Wah[:, :, i, :, :]
                    dstv = Bm[:, :, m, :, :]
                    if i == 0:
                        nc.vector.tensor_scalar(dstv, src, sc, None, ALU.mult)
                    else:
                        nc.vector.scalar_tensor_tensor(dstv, src, sc, dstv,
                                                       ALU.mult, ALU.add)
            B2 = pe.tile([128, NST, 4, 2, 9], fp32, tag="B2")
            B3 = pe.tile([128, NST, 4, 2, 9], fp32, tag="B3")
            Et = pe.tile([128, NST, 4, 2, 9], fp32, tag="Et")
            EU = pe.tile([128, NST, 4, 2, 9], fp32, tag="EUt")
            fl3 = lambda ap: ap.rearrange("p t m e -> p (t m) e")
            for g0 in (0, 16):
                g1 = g0 + 16
                cmm((64,), fl3(Bm[:, g0:g1, :, 0, :]), fl3(Bm[:, g0:g1, :, 1, :]),
                    fl3(Bm[:, g0:g1, :, 0, :]), fl3(Bm[:, g0:g1, :, 1, :]),
                    fl3(B2[:, g0:g1, :, 0, :]), fl3(B2[:, g0:g1, :, 1, :]))
                cmm((64,), fl3(B2[:, g0:g1, :, 0, :]), fl3(B2[:, g0:g1, :, 1, :]),
                    fl3(Bm[:, g0:g1, :, 0, :]), fl3(Bm[:, g0:g1, :, 1, :]),
                    fl3(B3[:, g0:g1, :, 0, :]), fl3(B3[:, g0:g1, :, 1, :]))
            flat = lambda t: t[:].rearrange("p t m r e -> p (t m r e)")
            nc.vector.scalar_tensor_tensor(flat(Et), flat(B2), 0.5, flat(Bm),
                                           ALU.mult, ALU.subtract)
            nc.vector.scalar_tensor_tensor(flat(Et), flat(B3), -1.0 / 6.0,
                                           flat(Et), ALU.mult, ALU.add)
            nc.vector.tensor_scalar(Et[:, :, :, 0, 0:9:4], Et[:, :, :, 0, 0:9:4],
                                    1.0, None, ALU.add)
            Ur_o = ure[:].rearrange("p t (m e) -> p t m e", m=4)
            Ui_o = uim[:].rearrange("p t (m e) -> p t m e", m=4)
            for g0 in (0, 16):
                g1 = g0 + 16
                cmm((64,), fl3(Et[:, g0:g1, :, 0, :]), fl3(Et[:, g0:g1, :, 1, :]),
                    fl3(Ur_o[:, g0:g1]), fl3(Ui_o[:, g0:g1]),
                    fl3(EU[:, g0:g1, :, 0, :]), fl3(EU[:, g0:g1, :, 1, :]))
            drain(eur[:].rearrange("p t (m e) -> p t m e", m=4),
                  EU[:, :, :, 0, :])
            drain(eui[:].rearrange("p t (m e) -> p t m e", m=4),
                  EU[:, :, :, 1, :])
        if DBG:
            nc.sync.dma_start(dbg("dbg_eur", [128, NST, 36]).ap(), eur[:])
            nc.sync.dma_start(dbg("dbg_eui", [128, NST, 36]).ap(), eui[:])

        # ================= phase G: final get_W + fp16 out ================
        with ExitStack() as ph:
            pg = ph.enter_context(tc.tile_pool(name="pg", bufs=1))
            wsG = pg.tile([128, NST, 10, 2, 9], fp32, tag="wsG")
            emit_getw(eur, eui, wsG[:])
            w16 = pg.tile([128, NST, 120], fp16, tag="w16")
            nc.scalar.activation(
                w16[:].rearrange("p t (c r e) -> p t c r e", c=10, r=2),
                wsG[:, :, :, :, 0:6], ACT.Copy)
            nc.sync.dma_start(d_out.ap(), w16[:])
        pcd.close()
        es.close()

    nc.compile()
    # memoize the BIR JSON: the axon lowering path calls to_json_bytes on
    # every (re-traced) call; the module is immutable after compile.
    data = nc.to_json_bytes()
    nc.to_json_bytes = lambda: data
    return nc, sorted(dbg_t)


# ----------------------------------------------------------------- host entry
def _get_prog():
    if "prog" not in _CACHE:
        _CACHE["prog"] = build_program()
    return _CACHE["prog"]


def make_in_maps(inputs):
    U_re = np.asarray(inputs["U_re"], np.float32)
    U_im = np.asarray(inputs["U_im"], np.float32)
    omega = np.asarray(inputs["omega"], np.float32)
    alpha = np.asarray(inputs["alpha"], np.float32)
    beta = np.asarray(inputs["beta"], np.float32)

    # omega slot weights: slots 0..7 = MK_LIST (k=0,2), slot 8 = sum_m k=1
    wdir = np.zeros((10, 9, 8), np.float32)
    wdag = np.zeros((10, 9, 8), np.float32)
    wcon = np.zeros((1, 9, 8), np.float32)
    for s, (m, kk) in enumerate(MK_LIST):
        wdir[:, s, :] = omega[:, 0:10, m, kk].T
        wdag[:, s, :] = omega[:, 11:21, m, kk].T
        wcon[0, s, :] = omega[:, 10, m, kk]
    wdir[:, 8, :] = omega[:, 0:10, :, 1].sum(axis=2).T
    wdag[:, 8, :] = omega[:, 11:21, :, 1].sum(axis=2).T
    wcon[0, 8, :] = omega[:, 10, :, 1].sum(axis=1)

    at = alpha.transpose(1, 2, 0).reshape(64, 8)           # [(j k), i]
    ab = at[(128 * np.arange(9)[:, None] + np.arange(128)[None, :]) // 18]
    ab = np.ascontiguousarray(ab, np.float32)              # [9, 128, 8]

    fvec = FVEC_BASE.copy()
    off, _ = _F["alpha_jk_i"]
    fvec[off:off + 512] = at.reshape(-1)
    off, _ = _F["betav"]
    fvec[off:off + 32] = (beta / 2.0).reshape(-1)
    fvec = fvec.reshape(1, -1)

    in_maps = []
    for b in range(NCORES):
        u16r = np.zeros((128, NST, 36), np.float16)
        u16i = np.zeros((128, NST, 36), np.float16)
        u16r[IDX_P, IDX_T] = U_re[b].reshape(4096, 36)
        u16i[IDX_P, IDX_T] = U_im[b].reshape(4096, 36)
        in_maps.append({"u16r": u16r, "u16i": u16i, "vrow": VROW,
                        "fvec": fvec, "wdir": wdir.reshape(10, 72),
                        "wdag": wdag.reshape(10, 72),
                        "wcon": wcon.reshape(1, 72), "ab": ab})
    return in_maps


def _det3(M):
    # [..., 3, 3] complex -> [...] det via cofactors
    a, b_, c = M[..., 0, 0], M[..., 0, 1], M[..., 0, 2]
    d, e, f = M[..., 1, 0], M[..., 1, 1], M[..., 1, 2]
    g, h, i = M[..., 2, 0], M[..., 2, 1], M[..., 2, 2]
    return a * (e * i - f * h) - b_ * (d * i - f * g) + c * (d * h - e * g)


def channel_dets(U_re, U_im):
    """det of each output W channel per site; det(E)=1 (B traceless), so
    dets of get_W(EU) equal those of get_W(U), computable from the input."""
    U = (U_re + 1j * U_im).astype(np.complex64)        # [B,8,8,8,8,4,3,3]
    d = _det3(U)                                       # [B,8,8,8,8,4]
    dets = np.empty((B, L, L, L, L, 10), np.complex64)
    ch = 0
    for m in range(D):
        for n_ in range(m + 1, D):
            dm, dn = d[..., m], d[..., n_]
            dets[..., ch] = (dm * np.roll(dn, -1, axis=1 + m)
                             * np.conj(np.roll(dm, -1, axis=1 + n_))
                             * np.conj(dn))
            ch += 1
    for m in range(D):
        prod = d[..., m].prod(axis=1 + m, keepdims=True)
        dets[..., 6 + m] = prod                        # line product, bcast
    return dets


FLAT_IDX = IDX_P * NST + IDX_T


def _assemble_one(out, results, dets, b):
    eye = np.eye(3, dtype=np.complex64)
    w = np.asarray(results[b]["wout"])                 # fp16 [128, 32, 120]
    arr = w.reshape(128 * NST, 120).take(FLAT_IDX, axis=0).astype(np.float32)
    arr = arr.reshape(L, L, L, L, 10, 2, 2, 3)
    rows = (arr[..., 0, :, :] + 1j * arr[..., 1, :, :])    # [...,10,2,3]
    base = np.empty((L, L, L, L, 10, 3, 3), np.complex64)
    base[..., 0:2, :] = rows
    a0, a1, a2 = rows[..., 0, 0], rows[..., 0, 1], rows[..., 0, 2]
    b0, b1, b2 = rows[..., 1, 0], rows[..., 1, 1], rows[..., 1, 2]
    r3 = base[..., 2, :]
    r3[..., 0] = a1 * b2 - a2 * b1
    r3[..., 1] = a2 * b0 - a0 * b2
    r3[..., 2] = a0 * b1 - a1 * b0
    np.conjugate(r3, out=r3)
    r3 *= dets[b][..., None]
    out[b, :, :, :, :, 0:10] = base
    out[b, :, :, :, :, 10] = eye
    out[b, :, :, :, :, 11:21] = np.conj(np.swapaxes(base, -1, -2))


def assemble_output(results, dets):
    out = np.empty((B, L, L, L, L, NVAR, 3, 3), np.complex64)
    from concurrent.futures import ThreadPoolExecutor
    if "pool" not in _CACHE:
        _CACHE["pool"] = ThreadPoolExecutor(NCORES)
    futs = [_CACHE["pool"].submit(_assemble_one, out, results, dets, b)
            for b in range(NCORES)]
    for f in futs:
        f.result()
    return out


def _build_fast_runner(nc):
    """Same execute path run_bass_kernel_spmd takes under axon
    (bass2jax.run_bass_via_pjrt), but the jitted shard_map callable is built
    ONCE and cached, so warm calls skip the per-call retrace / re-lower /
    compile-cache lookup that a fresh closure pays every time."""
    import jax as _jax
    import concourse.mybir as mybir
    from concourse import bass2jax
    from jax.experimental.shard_map import shard_map
    from jax.sharding import Mesh, PartitionSpec

    bass2jax.install_neuronx_cc_hook()
    assert nc.dbg_addr is None
    part_name = nc.partition_id_tensor.name if nc.partition_id_tensor else None

    in_names, out_names, out_avals, zero_outs = [], [], [], []
    for alloc in nc.m.functions[0].allocations:
        if not isinstance(alloc, mybir.MemoryLocationSet):
            continue
        name = alloc.memorylocations[0].name
        if alloc.kind == "ExternalInput":
            if name != part_name:
                in_names.append(name)
        elif alloc.kind == "ExternalOutput":
            out_names.append(name)
            shape = tuple(alloc.tensor_shape)
            dtype = mybir.dt.np(alloc.dtype)
            out_avals.append(_jax.core.ShapedArray(shape, dtype))
            zero_outs.append(np.zeros((NCORES * shape[0], *shape[1:]), dtype))
    n_params = len(in_names)
    all_names = tuple(in_names) + tuple(out_names)
    if part_name is not None:
        all_names = all_names + (part_name,)
    donate = tuple(range(n_params, n_params + len(out_names)))

    def _body(*args):
        operands = list(args)
        if part_name is not None:
            operands.append(bass2jax.partition_id_tensor())
        outs = bass2jax._bass_exec_p.bind(
            *operands, out_avals=tuple(out_avals), in_names=all_names,
            out_names=tuple(out_names), lowering_input_output_aliases=(),
            sim_require_finite=True, sim_require_nnan=True, nc=nc)
        return tuple(outs)

    mesh = Mesh(np.asarray(_jax.devices()[:NCORES]), ("core",))
    specs = (PartitionSpec("core"),) * (n_params + len(out_names))
    sharded = _jax.jit(
        shard_map(_body, mesh=mesh, in_specs=specs,
                  out_specs=(PartitionSpec("core"),) * len(out_names),
                  check_rep=False),
        donate_argnums=donate, keep_unused=True)

    # donated output buffers filled ON DEVICE (no 2 MB host->device upload)
    import jax.numpy as _jnp
    from jax.sharding import NamedSharding
    sh_core = NamedSharding(mesh, PartitionSpec("core"))
    zeros_fn = _jax.jit(
        lambda: tuple(_jnp.zeros(z.shape, z.dtype) for z in zero_outs),
        out_shardings=(sh_core,) * len(zero_outs))

    zcache = []

    def run(in_maps):
        concat_in = [
            np.concatenate([np.asarray(m[name]) for m in in_maps], axis=0)
            for name in in_names]
        zs = zcache.pop() if zcache else zeros_fn()
        out_arrs = sharded(*concat_in, *zs)
        zcache.append(zeros_fn())   # async; overlaps fetch + host assembly
        outs_np = [np.asarray(a).reshape(NCORES, *out_avals[i].shape)
                   for i, a in enumerate(out_arrs)]
        return [{name: outs_np[i][c] for i, name in enumerate(out_names)}
                for c in range(NCORES)]

    return run


def _run(nc, in_maps):
    if "fast_runner" not in _CACHE:
        try:
            _CACHE["fast_runner"] = _build_fast_runner(nc)
        except Exception:
            _CACHE["fast_runner"] = None
    fast = _CACHE["fast_runner"]
    if fast is not None:
        try:
            return fast(in_maps)
        except Exception:
            _CACHE["fast_runner"] = None
    from concourse.bass_utils import run_bass_kernel_spmd
    return run_bass_kernel_spmd(nc, in_maps,
                                core_ids=list(range(NCORES))).results


def kernel(**inputs):
    nc, _dbg = _get_prog()
    in_maps = make_in_maps(inputs)
    dets = channel_dets(np.asarray(inputs["U_re"], np.float32),
                        np.asarray(inputs["U_im"], np.float32))
    results = _run(nc, in_maps)
    _CACHE["last_results"] = results
    return assemble_output(results, dets)
